# revision 9
# baseline (speedup 1.0000x reference)
"""DeepFactorRNN Trainium2 kernel.

Computes, for x = X.reshape(-1, F):
  mus    = sum_j(relu(LSTM2g(LSTM1g(x))) @ aff_W.T + aff_b)_j
  sigmas = softplus(relu(LSTM2n(LSTM1n(x))) @ noise_W.T + noise_b) + 1e-6
where each LSTM is a single step from zero state (so the forget gate is
unused and c = sigmoid(i)*tanh(g), h = sigmoid(o)*tanh(c)).

Strategy (8 NeuronCores, data parallel over the 32768 flattened rows):
 - Rows live on the matmul free dim; features/gates on partitions, so the
   whole network is transpose-free. X is transposed/cast on host.
 - f-gates are dropped from all weight matrices (25% matmul savings).
 - The aff linear + sum collapses to one dot with w_mu = aff_W.sum(0).
 - bf16 matmul operands, fp32 PSUM accumulation, fp32 activation math.
 - ACT keeps only the 3 gate table passes per chunk (its hard floor);
   tanh(c) runs on the DVE (|c|<=1) as fitted odd polys:
     global branch: c*(PD0 + PD1*c^2)   (2 tensor_tensor + 1 tensor_scalar)
     noise branch:  c*(AB0 + AB1*|c|)   (1 tensor_tensor + 2 tensor_scalar,
       using the abs_max/max alu op; the noise output has ~6x more error
       budget, and for layer 1 the relu folds into the max op with a final
       relu on h since sign(h) == sign(c))
 - Two adjacent 128-unit chunks share one [128, 2*RT] SBUF tile for all
   activation outputs and DVE ops, halving the DVE per-instruction
   overhead (~80-100ns each). PSUM tiles stay [128, RT] (bank limit).
 - Emission is software-pipelined with a one-tile skew: tile t's layer-0
   work (ACT-heavy) interleaves with tile t-1's layer-1 work (PE-heavy).
 - Tail row-sums for mu and sigma share one PSUM tile (sigma lands at
   partition 32 via tile_position); staging copies run on the DVE except
   the last tile's, which uses the then-idle ACT. The last tile's noise
   layer-1 group is emitted unpaired to shorten the pipeline drain.
 - The noise branch is computed first at startup (its weights are 4x
   smaller), so the PE starts ~2us earlier while w0g still loads.
 - The constant aff bias and the softplus epilogue fold on the host.
"""

from functools import partial

import numpy as np
import ml_dtypes

BF16 = ml_dtypes.bfloat16

NCORES = 8
NTS, NPER, F = 128, 256, 128
GH, NH = 512, 256
ROWS = NTS * NPER            # 32768
RPC = ROWS // NCORES         # 4096 rows per core
RT = 1024                    # rows per tile
NT = RPC // RT               # 4 tiles per core
HALF = 512                   # matmul moving max: one PSUM bank of fp32

# tanh(x) ~= x*(PD0 + PD1*x^2), least-squares fit over the empirical
# c = sigmoid(i)*tanh(g) distribution of all four LSTM layers
PD0, PD1 = 0.9925682, -0.26160714
# tanh(x) ~= x*(AB0 + AB1*x) for x >= 0, fit over the noise layer-1
# relu'd c distribution (used with the relu folded into the max op)
AB0, AB1 = 1.0060241, -0.09650258

_CACHE = {}


def _build_program():
    import concourse.bacc as bacc
    import concourse.tile as tile
    from concourse import mybir

    dt = mybir.dt
    AFT = mybir.ActivationFunctionType
    ALU = mybir.AluOpType

    nc = bacc.Bacc("TRN2", target_bir_lowering=False, debug=False,
                   num_devices=NCORES)

    # ---- DRAM I/O ----
    d_xT = nc.dram_tensor("xT", [F, RPC], dt.bfloat16, kind="ExternalInput")
    d_w0g = nc.dram_tensor("w0g", [F, 3 * GH], dt.bfloat16, kind="ExternalInput")
    d_w1g = nc.dram_tensor("w1g", [GH, 3 * GH], dt.bfloat16, kind="ExternalInput")
    d_w0n = nc.dram_tensor("w0n", [F, 3 * NH], dt.bfloat16, kind="ExternalInput")
    d_w1n = nc.dram_tensor("w1n", [NH, 3 * NH], dt.bfloat16, kind="ExternalInput")
    d_wmu = nc.dram_tensor("wmu", [128, GH // 128], dt.bfloat16, kind="ExternalInput")
    d_wsig = nc.dram_tensor("wsig", [128, NH // 128], dt.bfloat16, kind="ExternalInput")
    d_bg0 = nc.dram_tensor("bg0", [128, 3 * GH // 128], dt.float32, kind="ExternalInput")
    d_bg1 = nc.dram_tensor("bg1", [128, 3 * GH // 128], dt.float32, kind="ExternalInput")
    d_bn0 = nc.dram_tensor("bn0", [128, 3 * NH // 128], dt.float32, kind="ExternalInput")
    d_bn1 = nc.dram_tensor("bn1", [128, 3 * NH // 128], dt.float32, kind="ExternalInput")
    d_mus = nc.dram_tensor("mus_o", [1, RPC], dt.float32, kind="ExternalOutput")
    d_zs = nc.dram_tensor("zs_o", [1, RPC], dt.float32, kind="ExternalOutput")

    CG = GH // 128   # 4 chunks for global hidden
    CN = NH // 128   # 2 chunks for noise hidden

    with tile.TileContext(nc) as tc:
        with (
            tc.tile_pool(name="wp", bufs=1) as wp,
            tc.tile_pool(name="gp", bufs=2) as gp,
            tc.tile_pool(name="hp", bufs=4) as hp,
            tc.tile_pool(name="pp", bufs=4, space="PSUM") as pp,
        ):
            # ---- resident loads: noise weights + tile-0 x first so the
            # PE can start on the noise branch while w0g still loads ----
            w0n = wp.tile([F, 3 * NH], dt.bfloat16, name="w0n_sb")
            nc.sync.dma_start(out=w0n, in_=d_w0n[:, :])
            xTs = [wp.tile([F, RT], dt.bfloat16, name=f"xT_sb{t}")
                   for t in range(NT)]
            nc.sync.dma_start(out=xTs[0], in_=d_xT[:, 0:RT])
            bn0 = wp.tile([128, 3 * CN], dt.float32, name="bn0_sb")
            nc.sync.dma_start(out=bn0, in_=d_bn0[:, :])
            w0g = wp.tile([F, 3 * GH], dt.bfloat16, name="w0g_sb")
            nc.sync.dma_start(out=w0g, in_=d_w0g[:, :])
            bg0 = wp.tile([128, 3 * CG], dt.float32, name="bg0_sb")
            nc.sync.dma_start(out=bg0, in_=d_bg0[:, :])
            w1n = [wp.tile([128, 3 * NH], dt.bfloat16, name=f"w1n_sb{k}")
                   for k in range(CN)]
            for k in range(CN):
                nc.sync.dma_start(out=w1n[k], in_=d_w1n[k * 128:(k + 1) * 128, :])
            bn1 = wp.tile([128, 3 * CN], dt.float32, name="bn1_sb")
            nc.sync.dma_start(out=bn1, in_=d_bn1[:, :])
            w1g = [wp.tile([128, 3 * GH], dt.bfloat16, name=f"w1g_sb{k}")
                   for k in range(CG)]
            for k in range(CG):
                nc.sync.dma_start(out=w1g[k], in_=d_w1g[k * 128:(k + 1) * 128, :])
            bg1 = wp.tile([128, 3 * CG], dt.float32, name="bg1_sb")
            nc.sync.dma_start(out=bg1, in_=d_bg1[:, :])
            for t in range(1, NT):
                nc.sync.dma_start(out=xTs[t], in_=d_xT[:, t * RT:(t + 1) * RT])
            wmu = wp.tile([128, CG], dt.bfloat16, name="wmu_sb")
            nc.sync.dma_start(out=wmu, in_=d_wmu[:, :])
            wsig = wp.tile([128, CN], dt.bfloat16, name="wsig_sb")
            nc.sync.dma_start(out=wsig, in_=d_wsig[:, :])

            def layer_group(t, C, rhs, w_list, b_sb, out_tag, form, relu,
                            hbufs, paired=True, htag=None):
                """One full LSTM step for RT rows. Emits per-pair (or
                per-chunk if not paired) thunks; returns (thunks, outs)
                where outs[j] = (tile, colbase) for chunk j's h values.
                rhs is a list over k-chunks of (tile, colbase)."""
                nk = len(rhs)
                W = 2 if paired else 1
                P = C // W
                outs = [None] * C

                def group(p):
                    acts = []
                    for hf in range(W):
                        c = p * W + hf
                        ps = []
                        for gi in range(3):  # i, g, o
                            pt = pp.tile([128, RT], dt.float32, tag="ps", bufs=4,
                                         name=f"p_{out_tag}_{t}_{c}_{gi}")
                            mcol = (gi * C + c) * 128
                            for k in range(nk):
                                rtile, rbase = rhs[k]
                                for h in range(RT // HALF):
                                    hs = slice(rbase + h * HALF,
                                               rbase + (h + 1) * HALF)
                                    nc.tensor.matmul(
                                        pt[:, h * HALF:(h + 1) * HALF],
                                        w_list[k][:, mcol:mcol + 128],
                                        rtile[:, hs],
                                        start=(k == 0), stop=(k == nk - 1),
                                    )
                            ps.append(pt)
                        acts.append(ps)
                    ti = gp.tile([128, W * RT], dt.bfloat16, tag="ti" + out_tag[-1],
                                 bufs=2, name=f"ti_{out_tag}_{t}_{p}")
                    tg = gp.tile([128, W * RT], dt.bfloat16, tag="tg" + out_tag[-1],
                                 bufs=2, name=f"tg_{out_tag}_{t}_{p}")
                    to = gp.tile([128, W * RT], dt.bfloat16, tag="to" + out_tag[-1],
                                 bufs=3, name=f"to_{out_tag}_{t}_{p}")
                    for hf in range(W):
                        c = p * W + hf
                        fs = slice(hf * RT, (hf + 1) * RT)
                        pi, pg, po = acts[hf]
                        nc.scalar.activation(ti[:, fs], pi, AFT.Sigmoid,
                                             bias=b_sb[:, c:c + 1])
                        nc.scalar.activation(tg[:, fs], pg, AFT.Tanh,
                                             bias=b_sb[:, C + c:C + c + 1])
                        nc.scalar.activation(to[:, fs], po, AFT.Sigmoid,
                                             bias=b_sb[:, 2 * C + c:2 * C + c + 1])
                    cc = gp.tile([128, W * RT], dt.bfloat16, tag="cc", bufs=2,
                                 name=f"cc_{out_tag}_{t}_{p}")
                    nc.vector.tensor_mul(cc, ti, tg)
                    th = gp.tile([128, W * RT], dt.bfloat16, tag="th", bufs=2,
                                 name=f"th_{out_tag}_{t}_{p}")
                    if form == "sq":
                        if relu:
                            nc.vector.tensor_scalar_max(cc, cc, 0.0)
                        tq = gp.tile([128, W * RT], dt.bfloat16, tag="pta",
                                     bufs=2, name=f"tq_{out_tag}_{t}_{p}")
                        nc.vector.tensor_mul(tq, cc, cc)
                        qq = gp.tile([128, W * RT], dt.bfloat16, tag="ptb",
                                     bufs=2, name=f"qq_{out_tag}_{t}_{p}")
                        nc.vector.tensor_scalar(qq, tq, PD1, PD0, op0=ALU.mult,
                                                op1=ALU.add)
                        nc.vector.tensor_mul(th, qq, cc)
                    else:
                        # c*(AB0 + AB1*max(c,0)): the layer-1 relu folds into
                        # the max (h keeps c's sign; relu on h at the end)
                        u = gp.tile([128, W * RT], dt.bfloat16, tag="pta",
                                    bufs=2, name=f"u_{out_tag}_{t}_{p}")
                        nc.vector.tensor_scalar(
                            u, cc, 0.0, AB1, op0=ALU.max, op1=ALU.mult)
                        w = gp.tile([128, W * RT], dt.bfloat16, tag="ptb",
                                    bufs=2, name=f"w_{out_tag}_{t}_{p}")
                        nc.vector.tensor_scalar_add(w, u, AB0)
                        nc.vector.tensor_mul(th, w, cc)
                    h = hp.tile([128, W * RT], dt.bfloat16, tag=htag or out_tag,
                                bufs=hbufs, name=f"h_{out_tag}_{t}_{p}")
                    nc.vector.tensor_mul(h, to, th)
                    if form == "abs" and relu:
                        nc.vector.tensor_scalar_max(h, h, 0.0)
                    for hf in range(W):
                        outs[p * W + hf] = (h, hf * RT)

                return [partial(group, p) for p in range(P)], outs

            def tail_thunks(t, r1g, r1n, last=False):
                # single-row sums: mu[row] = wmu . r1g[:, row] at partition 0,
                # sig[row] = wsig . r1n[:, row] at partition 32 of the same
                # PSUM tile -> one staging copy serves both outputs.
                box = {}

                def emit_mu():
                    pz = pp.tile([33, RT], dt.float32, tag="ps", bufs=4,
                                 name=f"pz_{t}")
                    box["pz"] = pz
                    for k in range(CG):
                        rtile, rbase = r1g[k]
                        for h in range(RT // HALF):
                            nc.tensor.matmul(
                                pz[0:1, h * HALF:(h + 1) * HALF],
                                wmu[:, k:k + 1],
                                rtile[:, rbase + h * HALF:rbase + (h + 1) * HALF],
                                start=(k == 0), stop=(k == CG - 1))

                def emit_sig():
                    pz = box["pz"]
                    for k in range(CN):
                        rtile, rbase = r1n[k]
                        for h in range(RT // HALF):
                            nc.tensor.matmul(
                                pz[32:33, h * HALF:(h + 1) * HALF],
                                wsig[:, k:k + 1],
                                rtile[:, rbase + h * HALF:rbase + (h + 1) * HALF],
                                start=(k == 0), stop=(k == CN - 1),
                                tile_position=(0, 32))
                    st = gp.tile([33, RT], dt.float32, tag="st", bufs=2,
                                 name=f"st_{t}")
                    if last:
                        # ACT is idle during the drain; DVE is the critical path
                        nc.scalar.copy(st, pz)
                    else:
                        nc.vector.tensor_copy(st, pz)
                    nc.sync.dma_start(out=d_mus[:, t * RT:(t + 1) * RT],
                                      in_=st[0:1, :])
                    nc.sync.dma_start(out=d_zs[:, t * RT:(t + 1) * RT],
                                      in_=st[32:33, :])

                return emit_mu, emit_sig

            # Software pipeline with one-tile skew: tile t's layer-0 work
            # (ACT-heavy, PE-light) is emitted interleaved with tile t-1's
            # layer-1 work (PE-heavy, ACT-light), so both engine queues stay
            # dense and the PE never idles long enough to lose HAM warmth.
            light, heavy, tails = [], [], []
            for t in range(NT):
                lastt = t == NT - 1
                xrhs = [(xTs[t], 0)]
                b_th, h0n = layer_group(t, CN, xrhs, [w0n], bn0, "h0n",
                                        "sq", False, hbufs=2)
                a_th, h0g = layer_group(t, CG, xrhs, [w0g], bg0, "h0g",
                                        "sq", False, hbufs=4)
                c_th, r1g = layer_group(t, CG, h0g, w1g, bg1, "r1g",
                                        "sq", True, hbufs=6)
                d_th, r1n = layer_group(t, CN, h0n, w1n, bn1, "r1n",
                                        "abs", True, hbufs=2,
                                        paired=not lastt,
                                        htag="r1nL" if lastt else None)
                mu_th, sig_th = tail_thunks(t, r1g, r1n, last=lastt)
                light.append(b_th + a_th)
                heavy.append(c_th + d_th)
                tails.append([mu_th, sig_th])

            def interleave(xs, ys):
                out = []
                n = max(len(xs), len(ys))
                for i in range(n):
                    if i < len(xs):
                        out.append(xs[i])
                    if i < len(ys):
                        out.append(ys[i])
                return out

            # mu/sig tails have no consumers, so they are emitted a full
            # round after their r1 inputs: their matmuls are always
            # instantly ready and never head-of-line-block the PE FIFO
            for th in light[0]:
                th()
            for r in range(1, NT):
                stream = heavy[r - 1] + (tails[r - 2] if r >= 2 else [])
                for th in interleave(stream, light[r]):
                    th()
            # final drain: r1g pairs, then the unpaired r1n chunks with the
            # tile's tails slotted where their inputs are already ready
            fin = heavy[NT - 1]
            for th in fin[:2]:
                th()
            fin[2]()                  # r1n chunk 0
            tails[NT - 2][0]()        # old mu (ready)
            tails[NT - 1][0]()        # this tile's mu (r1g ready)
            fin[3]()                  # r1n chunk 1
            tails[NT - 2][1]()        # old sig + copy + dma
            tails[NT - 1][1]()        # final sig + ACT copy + dma

    nc.compile()
    return nc


def _pack_lstm_weights(W, b, H):
    """Drop the f gate; pack [i, g, o] along the output dim.
    Returns lhsT (K, 3H) bf16 and bias tile (128, 3H/128) f32."""
    idx = np.r_[0:H, 2 * H:3 * H, 3 * H:4 * H]
    Wp = W[idx]                      # (3H, K)
    bp = b[idx]                      # (3H,)
    lhsT = np.ascontiguousarray(Wp.T).astype(BF16)
    btile = np.ascontiguousarray(bp.reshape(3 * H // 128, 128).T).astype(np.float32)
    return lhsT, btile


def _make_in_maps(inputs):
    """Host-side packing: shard X, drop f-gates, fold aff into one dot.
    Returns (per-core input maps, summed aff bias, noise bias)."""
    X = np.asarray(inputs["X"], np.float32)
    g_Wih0 = np.asarray(inputs["g_Wih0"], np.float32)
    g_b0 = np.asarray(inputs["g_b0"], np.float32)
    g_Wih1 = np.asarray(inputs["g_Wih1"], np.float32)
    g_b1 = np.asarray(inputs["g_b1"], np.float32)
    aff_W = np.asarray(inputs["aff_W"], np.float32)
    aff_b = np.asarray(inputs["aff_b"], np.float32)
    n_Wih0 = np.asarray(inputs["n_Wih0"], np.float32)
    n_b0 = np.asarray(inputs["n_b0"], np.float32)
    n_Wih1 = np.asarray(inputs["n_Wih1"], np.float32)
    n_b1 = np.asarray(inputs["n_b1"], np.float32)
    noise_W = np.asarray(inputs["noise_W"], np.float32)
    noise_b = np.asarray(inputs["noise_b"], np.float32)

    w0g, bg0 = _pack_lstm_weights(g_Wih0, g_b0, GH)
    w1g, bg1 = _pack_lstm_weights(g_Wih1, g_b1, GH)
    w0n, bn0 = _pack_lstm_weights(n_Wih0, n_b0, NH)
    w1n, bn1 = _pack_lstm_weights(n_Wih1, n_b1, NH)

    wm = aff_W.sum(axis=0)                     # (GH,)
    wmu = np.ascontiguousarray(wm.reshape(GH // 128, 128).T).astype(BF16)
    b_mu = float(aff_b.sum())
    ws = noise_W[0]                            # (NH,)
    wsig = np.ascontiguousarray(ws.reshape(NH // 128, 128).T).astype(BF16)
    b_sig = float(noise_b[0])

    Xf = X.reshape(ROWS, F)
    shared = {
        "w0g": w0g, "w1g": w1g, "w0n": w0n, "w1n": w1n,
        "wmu": wmu, "wsig": wsig,
        "bg0": bg0, "bg1": bg1, "bn0": bn0, "bn1": bn1,
    }
    in_maps = []
    for c in range(NCORES):
        xc = np.ascontiguousarray(
            Xf[c * RPC:(c + 1) * RPC].T).astype(BF16)    # (F, RPC)
        in_maps.append({"xT": xc, **shared})
    return in_maps, b_mu, b_sig


def kernel(**inputs):
    from concourse.bass_utils import run_bass_kernel_spmd

    in_maps, b_mu, b_sig = _make_in_maps(inputs)
    if "nc" not in _CACHE:
        _CACHE["nc"] = _build_program()
    nc = _CACHE["nc"]

    res = run_bass_kernel_spmd(nc, in_maps, list(range(NCORES)))

    mus = np.empty(ROWS, np.float32)
    zs = np.empty(ROWS, np.float32)
    for c in range(NCORES):
        mus[c * RPC:(c + 1) * RPC] = res.results[c]["mus_o"][0]
        zs[c * RPC:(c + 1) * RPC] = res.results[c]["zs_o"][0]
    # device outputs the raw row sums; the constant aff bias, the softplus
    # epilogue over 32k scalars, and the +1e-6 epsilon fold on host
    mus = (mus + b_mu).reshape(NTS, NPER)
    sig = (np.logaddexp(0.0, zs + b_sig).astype(np.float32) + 1e-6).reshape(NTS, NPER)
    return mus, sig
